# revision 9
# baseline (speedup 1.0000x reference)
"""MiniGPT forward on 8 Trainium2 NeuronCores.

Sharding: token-split data parallelism. Core c owns the token half
s = c%2 (rows [s*TLOC, (s+1)*TLOC)) of sequence p = c//2. All weight
GEMMs, layernorms and the tied head are per-token and run fully local;
the only cross-core dependency is attention keys/values, exchanged once
per layer with an AllGather inside each core pair {2p, 2p+1}.

On-chip layout is feature-major: activations live as [128, n_tiles,
TLOC] SBUF tiles (feature on partitions, tokens on the free dim), so
every GEMM contraction dim is on partitions and weights stream in
pre-transposed (host-side) as [128, k_tile, out_features] bf16.
"""

import numpy as np
import ml_dtypes

P = 128

FULL_CFG = dict(B=4, T=1024, D=1024, H=16, HD=64, L=4, FF=4096, V=32000, NC=8,
                VCH=500, EPS=1e-5)


def _derived(cfg):
    d = dict(cfg)
    d["TLOC"] = cfg["B"] * cfg["T"] // cfg["NC"]     # tokens per core
    d["DT"] = cfg["D"] // P                          # d-model tiles
    d["QT"] = 3 * cfg["D"] // P                      # qkv output tiles
    d["KVT"] = 2 * cfg["D"] // P                     # k+v tiles
    d["FT"] = cfg["FF"] // P                         # ffn hidden tiles
    d["KTT"] = cfg["T"] // P                         # key tiles (full seq)
    d["MT"] = d["TLOC"] // P                         # token tiles per core
    d["NVC"] = cfg["V"] // cfg["VCH"]                # head vocab chunks
    d["HPT"] = P // cfg["HD"]                        # heads per 128-tile
    assert cfg["T"] % (2 * P) == 0 and d["TLOC"] * cfg["NC"] == cfg["B"] * cfg["T"]
    assert cfg["V"] % cfg["VCH"] == 0 and cfg["D"] % P == 0 and cfg["FF"] % P == 0
    assert d["TLOC"] <= 512, "single n-chunk design assumes TLOC <= 512"
    assert cfg["NC"] == 2 * cfg["B"], "one core pair per sequence"
    return d


def build_nc(cfg):
    import concourse.bass as bass
    import concourse.mybir as mybir
    import concourse.tile as tile
    from concourse import bacc
    from concourse.masks import make_identity

    f32 = mybir.dt.float32
    bf16 = mybir.dt.bfloat16
    AL = mybir.AluOpType
    AF = mybir.ActivationFunctionType

    c = _derived(cfg)
    B, T, D, H, HD, L, FF, V, NC = (cfg[k] for k in
                                    ("B", "T", "D", "H", "HD", "L", "FF", "V", "NC"))
    TLOC, DT, QT, KVT, FT, KTT, MT, NVC, HPT, VCH = (
        c[k] for k in ("TLOC", "DT", "QT", "KVT", "FT", "KTT", "MT", "NVC", "HPT", "VCH"))
    EPS = cfg["EPS"]
    QSCALE = 1.0 / np.sqrt(HD)
    groups = [[2 * p, 2 * p + 1] for p in range(NC // 2)]

    nc = bacc.Bacc("TRN2", target_bir_lowering=False, debug=False, num_devices=NC)

    def din(name, shape, dt=f32):
        return nc.dram_tensor(name, list(shape), dt, kind="ExternalInput")

    h0T = din("h0T", [P, DT, TLOC])
    wqkvT = din("wqkvT", [L, P, DT, 3 * D], bf16)
    wprojT = din("wprojT", [L, P, DT, D], bf16)
    wfc1T = din("wfc1T", [L, P, DT, FF], bf16)
    wfc2T = din("wfc2T", [L, P, FT, D], bf16)
    qkv_b = din("qkv_b", [L, P, QT])
    proj_b = din("proj_b", [L, P, DT])
    fc1_b = din("fc1_b", [L, P, FT])
    fc2_b = din("fc2_b", [L, P, DT])
    ln1_g = din("ln1_g", [L, P, DT])
    ln1_b = din("ln1_b", [L, P, DT])
    ln2_g = din("ln2_g", [L, P, DT])
    ln2_b = din("ln2_b", [L, P, DT])
    lnf_g = din("lnf_g", [P, DT])
    lnf_b = din("lnf_b", [P, DT])
    masks = din("masks", [P, KTT, TLOC], bf16)
    embT = din("embT", [P, DT, V], bf16)
    logits = nc.dram_tensor("logits", [TLOC, V], f32, kind="ExternalOutput")

    with tile.TileContext(nc) as tc:
        import contextlib
        ctx = contextlib.ExitStack()
        with ctx:
            persist = ctx.enter_context(tc.tile_pool(name="persist", bufs=1))
            ppool = ctx.enter_context(tc.tile_pool(name="ppool", bufs=8, space="PSUM"))
            wpool = ctx.enter_context(tc.tile_pool(name="wpool", bufs=2))
            bigpool = ctx.enter_context(tc.tile_pool(name="bigpool", bufs=1))
            actpool = ctx.enter_context(tc.tile_pool(name="actpool", bufs=2))
            qpool = ctx.enter_context(tc.tile_pool(name="qpool", bufs=1))
            kpool = ctx.enter_context(tc.tile_pool(name="kpool", bufs=1))
            p_pool = ctx.enter_context(tc.tile_pool(name="p_pool", bufs=1))
            vpool = ctx.enter_context(tc.tile_pool(name="vpool", bufs=4))
            vsrcpool = ctx.enter_context(tc.tile_pool(name="vsrcpool", bufs=2))
            rowpool = ctx.enter_context(tc.tile_pool(name="rowpool", bufs=4))
            tmppool = ctx.enter_context(tc.tile_pool(name="tmppool", bufs=2))
            parpool = ctx.enter_context(tc.tile_pool(name="parpool", bufs=2))
            embpool = ctx.enter_context(tc.tile_pool(name="embpool", bufs=2))
            outpool = ctx.enter_context(tc.tile_pool(name="outpool", bufs=2))
            drampool = ctx.enter_context(tc.tile_pool(name="drampool", bufs=2,
                                                      space="DRAM"))

            zero_col = persist.tile([P, 1], f32)
            nc.vector.memset(zero_col[:], 0.0)
            nc.const_aps.aps[(f32, 0.0)] = zero_col[:]
            eps_col = persist.tile([P, 1], f32)
            nc.vector.memset(eps_col[:], EPS)
            nc.const_aps.aps[(f32, EPS)] = eps_col[:]

            ones_f = persist.tile([P, P], f32)
            nc.vector.memset(ones_f[:], 1.0)
            ones_b = persist.tile([P, P], bf16)
            nc.vector.memset(ones_b[:], 1.0)
            ident_b = persist.tile([P, P], bf16)
            make_identity(nc, ident_b[:])

            masks_sb = persist.tile([P, KTT, TLOC], bf16)
            nc.sync.dma_start(masks_sb[:], masks.ap())

            h_sb = persist.tile([P, DT, TLOC], f32)
            nc.sync.dma_start(h_sb[:], h0T.ap())

            attout_sb = persist.tile([P, DT, TLOC], bf16)

            lnf_g_sb = persist.tile([P, DT], f32)
            nc.sync.dma_start(lnf_g_sb[:], lnf_g.ap())
            lnf_b_sb = persist.tile([P, DT], f32)
            nc.sync.dma_start(lnf_b_sb[:], lnf_b.ap())

            def layernorm(g_sb, b_sb, out_bf16_ap_fn, ntiles=DT):
                """h_sb [P, DT, TLOC] f32 -> bf16 tiles via out_bf16_ap_fn(t)."""
                sum_ps = ppool.tile([1, TLOC], f32, tag="ps", name="ln_sum")
                sq_ps = ppool.tile([1, TLOC], f32, tag="ps", name="ln_sq")
                for t in range(ntiles):
                    nc.tensor.matmul(sum_ps[:], ones_f[:, 0:1], h_sb[:, t, :],
                                     start=(t == 0), stop=(t == ntiles - 1))
                    hsq = tmppool.tile([P, TLOC], f32, tag="hsq", name="hsq")
                    nc.vector.tensor_tensor(hsq[:], h_sb[:, t, :], h_sb[:, t, :],
                                            AL.mult)
                    nc.tensor.matmul(sq_ps[:], ones_f[:, 0:1], hsq[:],
                                     start=(t == 0), stop=(t == ntiles - 1))
                mean = rowpool.tile([1, TLOC], f32, tag="row", name="mean")
                nc.vector.tensor_scalar_mul(mean[:], sum_ps[:], 1.0 / D)
                var = rowpool.tile([1, TLOC], f32, tag="row", name="var")
                nc.vector.tensor_scalar_mul(var[:], sq_ps[:], 1.0 / D)
                m2 = rowpool.tile([1, TLOC], f32, tag="row", name="m2")
                nc.vector.tensor_tensor(m2[:], mean[:], mean[:], AL.mult)
                nc.vector.tensor_sub(var[:], var[:], m2[:])
                std = rowpool.tile([1, TLOC], f32, tag="row", name="std")
                nc.scalar.activation(std[:], var[:], AF.Sqrt, bias=EPS)
                rstd = rowpool.tile([1, TLOC], f32, tag="row", name="rstd")
                nc.vector.reciprocal(rstd[:], std[:])
                mean_bc = ppool.tile([P, TLOC], f32, tag="ps", name="mean_bc")
                nc.tensor.matmul(mean_bc[:], ones_f[0:1, :], mean[:],
                                 start=True, stop=True)
                rstd_bc = ppool.tile([P, TLOC], f32, tag="ps", name="rstd_bc")
                nc.tensor.matmul(rstd_bc[:], ones_f[0:1, :], rstd[:],
                                 start=True, stop=True)
                for t in range(ntiles):
                    tmp = tmppool.tile([P, TLOC], f32, tag="lntmp", name="lntmp")
                    nc.vector.tensor_sub(tmp[:], h_sb[:, t, :], mean_bc[:])
                    nc.vector.tensor_tensor(tmp[:], tmp[:], rstd_bc[:], AL.mult)
                    nc.vector.tensor_scalar(out_bf16_ap_fn(t), tmp[:],
                                            g_sb[:, t:t + 1], b_sb[:, t:t + 1],
                                            AL.mult, AL.add)

            def load_par(src, l, width):
                t = parpool.tile([P, width], f32, tag=f"par{width}", name="par")
                nc.sync.dma_start(t[:], src.ap()[l])
                return t

            for l in range(L):
                g1 = load_par(ln1_g, l, DT)
                b1 = load_par(ln1_b, l, DT)
                a_in = actpool.tile([P, DT, TLOC], bf16, tag="a_in", name="a_in1")
                layernorm(g1, b1, lambda t: a_in[:, t, :])

                # ---- qkv GEMM: out tiles 0..DT-1 -> q_sb, DT..QT-1 -> kvout
                qb = load_par(qkv_b, l, QT)
                q_sb = qpool.tile([P, DT, TLOC], bf16, name="q_sb")
                kvout = bigpool.tile([P, FT, TLOC], bf16, tag="big", name="kvout")
                wq_ap = wqkvT.ap()[l]  # [P, DT, 3D]
                CH = 4 if QT % 4 == 0 else QT  # m-tiles per weight chunk
                for ch in range(QT // CH):
                    wt = wpool.tile([P, DT, CH * P], bf16, tag="w", name="wq")
                    nc.sync.dma_start(wt[:], wq_ap[:, :, ch * CH * P:(ch + 1) * CH * P])
                    for m in range(CH):
                        mt = ch * CH + m
                        ps = ppool.tile([P, TLOC], f32, tag="ps", name="qkv_ps")
                        for k in range(DT):
                            nc.tensor.matmul(ps[:], wt[:, k, m * P:(m + 1) * P],
                                             a_in[:, k, :],
                                             start=(k == 0), stop=(k == DT - 1))
                        scale = QSCALE if mt < DT else 1.0
                        dst = q_sb[:, mt, :] if mt < DT else kvout[:, mt - DT, :]
                        nc.vector.tensor_scalar(dst, ps[:], qb[:, mt:mt + 1],
                                                scale, AL.add, AL.mult)

                # ---- KV exchange within the pair
                kv_in = drampool.tile([KVT, P, TLOC], bf16, tag="bin", name="kv_in")
                nc.sync.dma_start(kv_in[:].rearrange("t p c -> p t c"),
                                  kvout[:, 0:KVT, :])
                kv_out = drampool.tile([2, KVT, P, TLOC], bf16, tag="bout",
                                       name="kv_out")
                nc.gpsimd.collective_compute(
                    "AllGather", AL.bypass, replica_groups=groups,
                    ins=[kv_in[:].opt()], outs=[kv_out[:].opt()])

                k_sb = kpool.tile([P, 2, DT, TLOC], bf16, name="k_sb")
                for sd in range(2):
                    nc.sync.dma_start(
                        k_sb[:, sd],
                        kv_out[sd, 0:DT, :, :].rearrange("t p c -> p t c"))

                # ---- attention, head pairs
                for i in range(H // 2):
                    vA = vpool.tile([P, KTT, HD + 1], bf16, tag="vaug", name="vA")
                    vB = vpool.tile([P, KTT, HD + 1], bf16, tag="vaug", name="vB")
                    nc.vector.memset(vA[:, :, HD:HD + 1], 1.0)
                    nc.vector.memset(vB[:, :, HD:HD + 1], 1.0)
                    for sd in range(2):
                        vsrc = vsrcpool.tile([P, TLOC], bf16, tag="vsrc",
                                             name="vsrc")
                        nc.sync.dma_start(vsrc[:], kv_out[sd, DT + i, :, :])
                        for cc in range(MT):
                            j = sd * (KTT // 2) + cc
                            tp = ppool.tile([P, P], bf16, tag="ps", name="vtp")
                            nc.tensor.transpose(tp[:], vsrc[:, cc * P:(cc + 1) * P],
                                                ident_b[:])
                            nc.scalar.copy(vA[:, j, 0:HD], tp[:, 0:HD])
                            nc.scalar.copy(vB[:, j, 0:HD], tp[:, HD:P])
                    for h in (2 * i, 2 * i + 1):
                        hb = (h % HPT) * HD
                        ht = h // HPT
                        vaug = vA if h % HPT == 0 else vB
                        p_t = p_pool.tile([P, KTT, TLOC], bf16, tag="P", name="p_t")
                        for j in range(KTT):
                            sd, jj = j // (KTT // 2), j % (KTT // 2)
                            sp = ppool.tile([P, TLOC], f32, tag="ps", name="s_ps")
                            nc.tensor.matmul(
                                sp[:],
                                k_sb[hb:hb + HD, sd, ht, jj * P:(jj + 1) * P],
                                q_sb[hb:hb + HD, ht, :], start=True, stop=True)
                            nc.scalar.activation(p_t[:, j, :], sp[:], AF.Exp)
                            nc.vector.tensor_tensor(p_t[:, j, :], p_t[:, j, :],
                                                    masks_sb[:, j, :], AL.mult)
                        ap = ppool.tile([P, TLOC], f32, tag="ps", name="av_ps")
                        for j in range(KTT):
                            nc.tensor.matmul(ap[0:HD + 1, :], vaug[:, j, :],
                                             p_t[:, j, :],
                                             start=(j == 0), stop=(j == KTT - 1))
                        rec = rowpool.tile([1, TLOC], f32, tag="row", name="rec")
                        nc.vector.reciprocal(rec[:], ap[HD:HD + 1, :])
                        recb = rowpool.tile([1, TLOC], bf16, tag="rowb", name="recb")
                        nc.vector.tensor_copy(recb[:], rec[:])
                        rb = ppool.tile([P, TLOC], f32, tag="ps", name="rb_ps")
                        nc.tensor.matmul(rb[0:HD, :], ones_b[0:1, 0:HD], recb[:],
                                         start=True, stop=True)
                        rb_sb = vsrcpool.tile([HD, TLOC], f32, tag="rb_sb",
                                              name="rb_sb")
                        nc.scalar.copy(rb_sb[:], rb[0:HD, :])
                        nc.vector.tensor_tensor(attout_sb[hb:hb + HD, ht, :],
                                                ap[0:HD, :], rb_sb[:], AL.mult)

                # ---- proj GEMM + residual
                pb = load_par(proj_b, l, DT)
                CHP = 4 if DT % 4 == 0 else DT
                for ch in range(DT // CHP):
                    wt = wpool.tile([P, DT, CHP * P], bf16, tag="w", name="wproj")
                    nc.sync.dma_start(
                        wt[:], wprojT.ap()[l][:, :, ch * CHP * P:(ch + 1) * CHP * P])
                    for m in range(CHP):
                        mt = ch * CHP + m
                        ps = ppool.tile([P, TLOC], f32, tag="ps", name="proj_ps")
                        for k in range(DT):
                            nc.tensor.matmul(ps[:], wt[:, k, m * P:(m + 1) * P],
                                             attout_sb[:, k, :],
                                             start=(k == 0), stop=(k == DT - 1))
                        nc.vector.scalar_tensor_tensor(h_sb[:, mt, :], ps[:],
                                                       pb[:, mt:mt + 1],
                                                       h_sb[:, mt, :],
                                                       AL.add, AL.add)

                # ---- mlp
                g2 = load_par(ln2_g, l, DT)
                b2 = load_par(ln2_b, l, DT)
                a2 = actpool.tile([P, DT, TLOC], bf16, tag="a_in", name="a_in2")
                layernorm(g2, b2, lambda t: a2[:, t, :])

                f1b = load_par(fc1_b, l, FT)
                mact = bigpool.tile([P, FT, TLOC], bf16, tag="big", name="mact")
                w1_ap = wfc1T.ap()[l]  # [P, DT, FF]
                CH1 = 4 if FT % 4 == 0 else FT
                for ch in range(FT // CH1):
                    wt = wpool.tile([P, DT, CH1 * P], bf16, tag="w", name="wfc1")
                    nc.sync.dma_start(wt[:],
                                      w1_ap[:, :, ch * CH1 * P:(ch + 1) * CH1 * P])
                    for m in range(CH1):
                        mt = ch * CH1 + m
                        ps = ppool.tile([P, TLOC], f32, tag="ps", name="fc1_ps")
                        for k in range(DT):
                            nc.tensor.matmul(ps[:], wt[:, k, m * P:(m + 1) * P],
                                             a2[:, k, :],
                                             start=(k == 0), stop=(k == DT - 1))
                        nc.scalar.activation(mact[:, mt, :], ps[:], AF.Gelu,
                                             bias=f1b[:, mt:mt + 1])

                f2b = load_par(fc2_b, l, DT)
                w2_ap = wfc2T.ap()[l]  # [P, FT, D]
                ps_fc2 = [ppool.tile([P, TLOC], f32, tag="ps", name=f"fc2_ps{m}")
                          for m in range(DT)]
                CH2 = 4 if FT % 4 == 0 else FT
                for ch in range(FT // CH2):
                    wt = wpool.tile([P, CH2, D], bf16, tag="w", name="wfc2")
                    nc.sync.dma_start(wt[:], w2_ap[:, ch * CH2:(ch + 1) * CH2, :])
                    for m in range(DT):
                        for k in range(CH2):
                            kt = ch * CH2 + k
                            nc.tensor.matmul(ps_fc2[m][:],
                                             wt[:, k, m * P:(m + 1) * P],
                                             mact[:, kt, :],
                                             start=(kt == 0), stop=(kt == FT - 1))
                for m in range(DT):
                    nc.vector.scalar_tensor_tensor(h_sb[:, m, :], ps_fc2[m][:],
                                                   f2b[:, m:m + 1], h_sb[:, m, :],
                                                   AL.add, AL.add)

            # ---- final layernorm + tied head
            af = actpool.tile([P, DT, TLOC], bf16, tag="a_in", name="a_f")
            layernorm(lnf_g_sb, lnf_b_sb, lambda t: af[:, t, :])

            for vc in range(NVC):
                ec = embpool.tile([P, DT, VCH], bf16, tag="emb", name="ec")
                nc.sync.dma_start(ec[:], embT.ap()[:, :, vc * VCH:(vc + 1) * VCH])
                for m in range(MT):
                    ps = ppool.tile([P, VCH], f32, tag="ps", name="head_ps")
                    for k in range(DT):
                        nc.tensor.matmul(ps[:], af[:, k, m * P:(m + 1) * P],
                                         ec[:, k, :],
                                         start=(k == 0), stop=(k == DT - 1))
                    ls = outpool.tile([P, VCH], f32, tag="lout", name="ls")
                    nc.scalar.copy(ls[:], ps[:])
                    nc.sync.dma_start(
                        logits.ap()[m * P:(m + 1) * P, vc * VCH:(vc + 1) * VCH],
                        ls[:])

    nc.compile()
    return nc


# ---------------------------------------------------------------------------
# host side
# ---------------------------------------------------------------------------

_CACHE = {}


def get_nc(cfg_key_and_cfg=None):
    cfg = FULL_CFG if cfg_key_and_cfg is None else cfg_key_and_cfg
    key = tuple(sorted(cfg.items()))
    if key not in _CACHE:
        _CACHE[key] = build_nc(cfg)
    return _CACHE[key]


def host_prep(inputs, cfg):
    """Build the per-core in_maps from full (unsharded) numpy inputs."""
    bf = ml_dtypes.bfloat16
    c = _derived(cfg)
    B, T, D, L, FF, V, NC = (cfg[k] for k in ("B", "T", "D", "L", "FF", "V", "NC"))
    TLOC, DT, QT, FT, KTT = (c[k] for k in ("TLOC", "DT", "QT", "FT", "KTT"))

    f = {k: np.asarray(v) for k, v in inputs.items()}
    x = f["x"].astype(np.int64)
    tok = f["tok_emb"].astype(np.float32)
    pos = f["pos_emb"].astype(np.float32)

    def wT_r(w, kdim, fdim):
        # [L, fdim, kdim] -> [L, 128, kdim/128, fdim] bf16
        wt = w.astype(np.float32).transpose(0, 2, 1)          # [L, kdim, fdim]
        wt = wt.reshape(L, kdim // P, P, fdim).transpose(0, 2, 1, 3)
        return np.ascontiguousarray(wt).astype(bf)

    def par_r(b, n):
        # [L, n*128] -> [L, 128, n]
        return np.ascontiguousarray(
            b.astype(np.float32).reshape(L, n, P).transpose(0, 2, 1))

    shared = {
        "wqkvT": wT_r(f["qkv_w"], D, 3 * D),
        "wprojT": wT_r(f["proj_w"], D, D),
        "wfc1T": wT_r(f["fc1_w"], D, FF),
        "wfc2T": wT_r(f["fc2_w"], FF, D),
        "qkv_b": par_r(f["qkv_b"], QT),
        "proj_b": par_r(f["proj_b"], DT),
        "fc1_b": par_r(f["fc1_b"], FT),
        "fc2_b": par_r(f["fc2_b"], DT),
        "ln1_g": par_r(f["ln1_g"], DT),
        "ln1_b": par_r(f["ln1_b"], DT),
        "ln2_g": par_r(f["ln2_g"], DT),
        "ln2_b": par_r(f["ln2_b"], DT),
        "lnf_g": np.ascontiguousarray(
            f["lnf_g"].astype(np.float32).reshape(DT, P).T),
        "lnf_b": np.ascontiguousarray(
            f["lnf_b"].astype(np.float32).reshape(DT, P).T),
        "embT": np.ascontiguousarray(
            tok.T.reshape(DT, P, V).transpose(1, 0, 2)).astype(bf),
    }

    in_maps = []
    for core in range(NC):
        p, s = core // 2, core % 2
        h0 = tok[x[p]] + pos[:T]                              # [T, D]
        h0 = h0[s * TLOC:(s + 1) * TLOC]                      # [TLOC, D]
        h0T = np.ascontiguousarray(
            h0.T.reshape(DT, P, TLOC).transpose(1, 0, 2)).astype(np.float32)
        kt_g = (np.arange(KTT * P).reshape(KTT, P))           # [KTT, P]
        q_g = s * TLOC + np.arange(TLOC)
        m = (kt_g[:, :, None] <= q_g[None, None, :])          # [KTT, P, TLOC]
        m = np.ascontiguousarray(m.transpose(1, 0, 2)).astype(bf)
        in_maps.append(dict(shared, h0T=h0T, masks=m))
    return in_maps


def assemble(results, cfg):
    c = _derived(cfg)
    B, T, V = cfg["B"], cfg["T"], cfg["V"]
    TLOC = c["TLOC"]
    out = np.empty((B, T, V), np.float32)
    for core, r in enumerate(results):
        p, s = core // 2, core % 2
        out[p, s * TLOC:(s + 1) * TLOC, :] = r["logits"]
    return out


def run(inputs, cfg=None, **run_kwargs):
    from concourse.bass_utils import run_bass_kernel_spmd
    cfg = cfg or FULL_CFG
    nc = get_nc(cfg)
    in_maps = host_prep(inputs, cfg)
    res = run_bass_kernel_spmd(nc, in_maps, core_ids=list(range(cfg["NC"])),
                               **run_kwargs)
    return assemble(res.results, cfg), res


def kernel(**inputs) -> np.ndarray:
    out, _ = run(inputs, FULL_CFG)
    return out
